# revision 13
# baseline (speedup 1.0000x reference)
"""Multi-head self-attention forward on 8 Trainium2 NeuronCores.

Problem: x[4,2048,512] -> qkv proj (w_qkv [512,1536]) -> 8-head attention
(head_dim 64) -> out proj (w_out [512,512] + b_out) -> y[4,2048,512].

Sharding: 8 shards = (batch b in 0..3) x (head-group hg in 0..1, 4 heads each).
Core c handles b=c//2, hg=c%2. Each core computes, for its batch and its 4
heads: qkv projection (only its heads' columns), attention, and the partial
output projection restricted to its heads' rows of w_out. Host sums the two
half-projections per batch and adds the bias.

On-device layout (all "T" tensors keep the contraction dim on partitions):
  xT   [512, 2048]   x[b] transposed (host-side transpose)
  qkT  4 tiles [128, 2048]: Q01, K01, Q23, K23 (2 heads stacked per tile:
       head A on partitions 0:64, head B on 64:128)
  v_aug 16 seq-tiles [128, 4*65]: per head 64 v columns + a ones column
       (the ones column makes the oT matmul also produce the softmax
       denominator as row 64 of its output)
  sT   [k, q] scores transposed -> exp (no max subtraction: |s|~N(0,1), safe
       in fp32) -> pT
  oT   v_aug.T @ pT = [65, q]: rows 0:64 unnormalized head output (d on
       partitions), row 64 = softmax denominator
  yproj y[q,c] per head = oT_head.T @ w2_head, scaled per-partition (q) by
       1/denom via tensor_scalar, summed over the 4 heads on DVE.
"""

import numpy as np

import concourse.bass as bass
import concourse.mybir as mybir
import concourse.tile as tile
from concourse import bacc

DIM = 512
NHEADS = 8
HD = 64
B = 4
SEQ = 2048
SCALE = HD ** -0.5

NCORES = 8
HPC = 4          # heads per core
QCH = 512        # q chunk (moving free dim)
NQC = SEQ // QCH # 4 q-chunks
KCH = 128        # k chunk (psum partition dim)
NKC = SEQ // KCH # 16 k-chunks
CCH = 128        # contraction chunk for projections
NCC = DIM // CCH # 4

F32 = mybir.dt.float32
F32R = mybir.dt.float32r

BF16 = mybir.dt.bfloat16
F16 = mybir.dt.float16
# matmul input dtype. fp16: 1 cycle/row like bf16 (same FP22 PE path, FWL
# eligible), but 10 mantissa bits instead of 7 — buys precision headroom
# that the fp8 DoubleRow AV path below spends. All values on these paths
# (x, w, q, k ~ N(0,1); exp(s-2) <= 45; unnormalized o <= ~2500) sit well
# inside fp16 range.
MMDT = F16

# exp bias: all exps compute exp(s*SCALE - EXPB). Keeps the fp8 pP values
# of the DoubleRow iterations inside TRN-e4m3 range (max 240, inf at 256);
# the softmax ratio cancels the constant. Applied uniformly so the
# denominator stays consistent across chunks.
EXPB = -2.0

PREFILL = False


def _mm(ap):
    return ap


def _emit_o(nc, oA, oB, vaug_t, pP, i, p, start, stop):
    """Accumulate the two kc chunks of pair-iteration i into oA/oB.

    pP packs both heads' exp'd scores: [A(kc) A(kc+1) B(kc) B(kc+1)],
    each QCH wide."""
    QCH = 512
    for hh, odst in ((0, oA), (1, oB)):
        for half in range(2):
            kc = 2 * i + half
            c0 = (2 * hh + half) * QCH
            nc.tensor.matmul(
                odst[:],
                _mm(vaug_t(kc)[:, 2 * p + hh, :]),
                _mm(pP[:, c0:c0 + QCH]),
                start=(start and half == 0), stop=(stop and half == 1),
                skip_group_check=True,
            )


def build_nc():
    nc = bacc.Bacc()

    xT_d = nc.dram_tensor("xt", [DIM, SEQ], MMDT, kind="ExternalInput")
    wperm_d = nc.dram_tensor("wperm", [DIM, 4 * 128], MMDT, kind="ExternalInput")
    wv_d = nc.dram_tensor("wv", [DIM, HPC * HD], MMDT, kind="ExternalInput")
    w2_d = nc.dram_tensor("w2", [HPC * HD, DIM], MMDT, kind="ExternalInput")
    y_d = nc.dram_tensor("y", [SEQ, DIM], BF16, kind="ExternalOutput")

    with tile.TileContext(nc) as tc:
        with (
            tc.tile_pool(name="const", bufs=1) as cpool,
            tc.tile_pool(name="big", bufs=1) as bigpool,
            tc.tile_pool(name="pt", bufs=4) as ptpool,
            tc.tile_pool(name="yacc", bufs=1) as yaccpool,
            tc.tile_pool(name="tmp", bufs=3) as tmppool,
            tc.tile_pool(name="small", bufs=2) as smallpool,
            tc.tile_pool(name="ps", bufs=1, space="PSUM") as ps,
        ):
            # ---- constants / inputs to SBUF ----
            # DMA lands fp32 in a staging tile; a DVE copy rounds into the
            # f32r tile the matmuls read (BIR f32r-rounding requirement).
            xTs = [cpool.tile([128, SEQ], MMDT, tag=f"xT{c}", name=f"xT{c}")
                   for c in range(NCC)]
            wps = [cpool.tile([128, 512], MMDT, tag=f"wp{c}", name=f"wp{c}")
                   for c in range(NCC)]
            wvs = [cpool.tile([128, HPC * HD], MMDT, tag=f"wv{c}", name=f"wv{c}")
                   for c in range(NCC)]
            w2s = [cpool.tile([128, DIM], MMDT, tag=f"w2{p}", name=f"w2{p}")
                   for p in range(2)]
            wscr = cpool.tile([128, 512], MMDT, tag="wscr")
            nc.gpsimd.memset(wscr[:], 0.0)
            ones4 = cpool.tile([128, HPC], F32, tag="ones4")
            nc.gpsimd.memset(ones4[:], 1.0)
            ones1 = cpool.tile([1, 1], F32, tag="ones1")
            nc.gpsimd.memset(ones1[:], 1.0)
            # preload the exp ACT table set early so the first real exp in
            # the attention phase doesn't stall the pipeline ~2.7us (the
            # PE gap there is what re-throttles HAM to K=4/8)
            selst = cpool.tile([33, 2], F32, tag="selst")
            nc.gpsimd.memset(selst[:], 0.0)
            nc.gpsimd.memset(selst[0:1, 0:1], 1.0)
            nc.gpsimd.memset(selst[32:33, 1:2], 1.0)
            sel = cpool.tile([33, 2], F32R, tag="sel")
            nc.vector.tensor_copy(sel[:], selst[:])
            zden = cpool.tile([33, QCH], F32, tag="zden")
            nc.gpsimd.memset(zden[:], 0.0)
            embias = cpool.tile([128, 1], F32, tag="embias")
            nc.gpsimd.memset(embias[:], EXPB)
            dummy = cpool.tile([1, 1], F32, tag="dummy")
            nc.scalar.activation(dummy[:], ones1[:],
                                 mybir.ActivationFunctionType.Exp)
            # PE warmup during the input-DMA wait: junk matmuls keep the PE
            # busy so the HAM clock-gate reaches K=8/8 (2.4 GHz) before
            # phase 1, instead of gambling on the free-running HAM phase
            # (observed ~1-in-7 runs entering a cold-clock regime, +35us).
            # 20 junk matmuls (~8.5us at the cold 1.2GHz clock): enough to
            # keep the PE busy through the whole input-DMA window, so the
            # HAM activity monitor never sees an idle MID window between
            # warmup and phase 1 (which would re-throttle the clock and run
            # phase 1 at 1.2GHz).
            wups = ps.tile([4, 512], F32, tag="y", bufs=2, name="wups")
            for _ in range(20):
                nc.tensor.matmul(wups[:], wscr[:, 0:4], wscr[:],
                                 start=True, stop=True,
                                 skip_group_check=True)

            # Input DMAs ordered by first use: phase 1a accumulates over c,
            # so interleaving (wperm chunk c, xT chunk c) lets its first
            # matmul start after two transfers instead of eight; everything
            # else (wv for the v fillers, late wperm cols, w2) follows.
            # Input DMAs ordered by first use: phase 1a needs wperm cols
            # 0:256 + the leading xT seq halves; wv (v fillers), the late
            # xT/wperm cols and w2 follow. All on the hardware sync queue —
            # gpsimd's software DGE is too slow for anything load-bearing.
            for c in range(NCC):
                nc.sync.dma_start(wps[c][:, 0:256],
                                  wperm_d[c * 128:(c + 1) * 128, 0:256])
            # xT leading halves split into quarters: each DMA descriptor
            # runs ~40GB/s on one engine, so finer slices double the
            # engine-level parallelism on the phase-1a critical path (and
            # its first matmuls only need the first seq quarter).
            for c in range(NCC):
                nc.sync.dma_start(xTs[c][:, 0:512],
                                  xT_d[c * 128:(c + 1) * 128, 0:512])
            for c in range(NCC):
                nc.sync.dma_start(xTs[c][:, 512:1024],
                                  xT_d[c * 128:(c + 1) * 128, 512:1024])
            for c in range(NCC):
                nc.sync.dma_start(wvs[c][:], wv_d[c * 128:(c + 1) * 128, :])
            for c in range(NCC):
                nc.sync.dma_start(xTs[c][:, 1024:SEQ],
                                  xT_d[c * 128:(c + 1) * 128, 1024:SEQ])
            for c in range(NCC):
                nc.sync.dma_start(wps[c][:, 256:512],
                                  wperm_d[c * 128:(c + 1) * 128, 256:512])
            for p in range(2):
                nc.sync.dma_start(w2s[p][:], w2_d[p * 128:(p + 1) * 128, :])

            def xT_c(c):
                return xTs[c]

            # ---- persistent intermediates ----
            qkTs = [bigpool.tile([128, SEQ], MMDT, tag=f"qkT{m}",
                                 name=f"qkT{m}") for m in range(4)]
            vaugs = [bigpool.tile([128, HPC * 65], MMDT, tag=f"vaug{st}",
                                  name=f"vaug{st}") for st in range(NKC)]
            oT = bigpool.tile([128, 2 * SEQ], MMDT, tag="oT")    # pair-packed
            yacc = yaccpool.tile([128, SEQ // 128 * DIM], F32, tag="yacc")
            # persistent score psum: 4 banks. Quarters are written by the
            # score matmuls and read by the exps; dependencies are tracked
            # at bank granularity within the tensor, so head A's banks
            # (0:1024) recycle independently of head B's (1024:2048).
            sps = ps.tile([128, 4 * QCH], F32, tag="s", bufs=1, name="sps")

            def qkT_blk(m):
                return qkTs[m]

            def vaug_t(kc):
                # [128, HPC, 65] view of seq-tile kc
                return vaugs[kc].rearrange("p (h e) -> p h e", e=65)

            # the ones columns of every vaug tile never change: write them
            # all once in the preamble (off the steady-state DVE path)
            for st in range(NKC):
                nc.vector.tensor_copy(
                    vaug_t(st)[:, :, 64:65],
                    ones4[:].rearrange("p (h o) -> p h o", o=1))

            def v_unit(st, tag, bufs):
                pv = ps.tile([128, HPC * HD], F32, tag=tag, bufs=bufs,
                             name="pv")
                for c in range(NCC):
                    nc.tensor.matmul(
                        pv[:],
                        _mm(xT_c(c)[:, st * 128:(st + 1) * 128]),
                        _mm(wvs[c][:]),
                        start=(c == 0), stop=(c == NCC - 1),
                        skip_group_check=True,
                    )
                vt = vaug_t(st)
                nc.vector.tensor_copy(
                    vt[:, :, 0:64], pv[:].rearrange("p (h d) -> p h d", d=HD)
                )

            def qk_unit(m, s2, tag, bufs):
                pp = ps.tile([128, 512], F32, tag=tag, bufs=bufs, name="pp")
                for c in range(NCC):
                    nc.tensor.matmul(
                        pp[:],
                        _mm(wps[c][:, m * 128:(m + 1) * 128]),
                        _mm(xT_c(c)[:, s2 * 512:(s2 + 1) * 512]),
                        start=(c == 0), stop=(c == NCC - 1),
                        skip_group_check=True,
                    )
                nc.vector.tensor_copy(qkTs[m][:, s2 * 512:(s2 + 1) * 512],
                                      pp[:])

            # ---- phase 1a (minimal serial prefix): Q01 for the first
            # q-block and K01 for the first half-seq; everything else
            # becomes attention filler so the exp stream starts as early as
            # possible ----
            # m=0 and m=1 use different bank-pairs of the persistent score
            # psum so m1's matmuls don't serialize behind m0's DVE
            # evacuation copy (bank-granular WAR)
            for m, units in ((0, (0,)), (1, (0, 1))):
                for s2 in units:
                    for c in range(NCC):
                        nc.tensor.matmul(
                            sps[:, (2 * m + (s2 & 1)) * 512:
                                (2 * m + (s2 & 1) + 1) * 512],
                            _mm(wps[c][:, m * 128:(m + 1) * 128]),
                            _mm(xT_c(c)[:, s2 * 512:(s2 + 1) * 512]),
                            start=(c == 0),
                            stop=(c == NCC - 1),
                            skip_group_check=True,
                        )
                    nc.vector.tensor_copy(
                        qkT_blk(m)[:, s2 * 512:(s2 + 1) * 512],
                        sps[:, (2 * m + (s2 & 1)) * 512:
                            (2 * m + (s2 & 1) + 1) * 512])

            # ---- phase 1b: first two v seq-tiles; the rest are filler ----
            for st in range(2):
                v_unit(st, "y", 2)

            # Filler order is load-bearing: block (p0, qc0) pops two units
            # per iteration (before the carried o-drain at i==0), and the
            # schedule is checked against every consumer deadline:
            # scores(i) need K01 cols 256i:256i+256 (qk(1,s2) by i=2*s2-2)
            # and o(j), drained at i=j+1, needs vaug tiles 2j, 2j+1 by then.
            # qk(0,1) (Q01 cols 512:1024, first used by block qc=1) rides
            # mid-list in a slot with no earlier deadline pressure.
            filler = [lambda st=st: v_unit(st, "y", 2) for st in (2, 3, 4, 5)]
            filler += [lambda: qk_unit(1, 2, "y", 2)]
            filler += [lambda st=st: v_unit(st, "y", 2) for st in (6,)]
            filler += [lambda: qk_unit(0, 1, "y", 2)]
            filler += [lambda st=st: v_unit(st, "y", 2) for st in (7,)]
            filler += [lambda: qk_unit(1, 3, "y", 2)]
            filler += [lambda st=st: v_unit(st, "y", 2)
                       for st in range(8, NKC)]
            filler += [lambda s2=s2: qk_unit(0, s2, "y", 2) for s2 in (2, 3)]
            filler += [lambda m=m, s2=s2: qk_unit(m, s2, "y", 2)
                       for m in (2, 3) for s2 in range(4)]
            if PREFILL:
                while filler:
                    filler.pop(0)()

            # ---- phase 2: attention + out-proj ----
            # kc chunks processed in pairs: one s psum tile [128, 1024] holds
            # scores for kc and kc+1 side by side (the same 128 k-partitions
            # map to different k-blocks per column half; exp is elementwise)
            # halving ACT instruction count. Two levels of software
            # pipelining keep the PE stream dense: within a block, s(i+1) is
            # emitted before o(i) so the PE never head-of-line blocks on
            # exp(i); across blocks, the out-projection of block n is spread
            # into the first pair-iterations of block n+1.
            NPAIR = NKC // 2

            def emit_rt(pend):
                p, qc, den2 = pend["p"], pend["qc"], pend["den2"]
                rt_ps = ps.tile([128, 2 * (QCH // 128)], F32, tag="y",
                                bufs=2, name="rt_ps")
                for j in range(QCH // 128):
                    # [2,128] -> [128,2] transpose as a K=33 selector matmul
                    # (is_transpose matmuls crash the HW; K=1 f32r fails an
                    # ISA check; heads sit on partitions 0 and 32 of den2)
                    nc.tensor.matmul(
                        rt_ps[:, 2 * j:2 * j + 2],
                        den2[:, j * 128:(j + 1) * 128],
                        sel[:],
                        start=True, stop=True, skip_group_check=True,
                    )
                dt_sb = smallpool.tile([128, 2 * (QCH // 128)], F32,
                                       tag="dt_sb")
                nc.vector.tensor_copy(dt_sb[:], rt_ps[:])
                rt = smallpool.tile([128, 2 * (QCH // 128)], F32, tag="rt_sb")
                nc.vector.reciprocal(rt[:], dt_sb[:])
                pend["rt"] = rt

            def emit_yproj_j(pend, j, tail=False):
                # Fused multiply-accumulate evacuation: ya accumulates the 4
                # head-group contributions via scalar_tensor_tensor
                # (out = in0*rt + in1) entirely on DVE; the final op writes
                # the bf16 output tile that is DMA'd out.
                p, qc, rt = pend["p"], pend["qc"], pend["rt"]
                qt = qc * (QCH // 128) + j
                yA = ps.tile([128, DIM], F32, tag="y", bufs=2, name="yA")[:]
                yB = ps.tile([128, DIM], F32, tag="y", bufs=2, name="yB")[:]
                oTp = oT[:, p * SEQ:(p + 1) * SEQ]
                nc.tensor.matmul(
                    yA,
                    _mm(oTp[0:64, qt * 128:(qt + 1) * 128]),
                    _mm(w2s[p][0:64, :]),
                    start=True, stop=True, skip_group_check=True,
                )
                nc.tensor.matmul(
                    yB,
                    _mm(oTp[64:128, qt * 128:(qt + 1) * 128]),
                    _mm(w2s[p][64:128, :]),
                    start=True, stop=True, skip_group_check=True,
                )
                ya = yacc[:, qt * DIM:(qt + 1) * DIM]
                MUL = mybir.AluOpType.mult
                ADD = mybir.AluOpType.add
                if p == 0:
                    nc.vector.tensor_scalar_mul(
                        ya, yA, rt[:, 2 * j:2 * j + 1])
                    nc.vector.scalar_tensor_tensor(
                        ya, yB, rt[:, 2 * j + 1:2 * j + 2], ya, MUL, ADD)
                else:
                    nc.vector.scalar_tensor_tensor(
                        ya, yA, rt[:, 2 * j:2 * j + 1], ya, MUL, ADD)
                    yo = smallpool.tile([128, DIM], BF16, tag="yout")
                    nc.vector.scalar_tensor_tensor(
                        yo[:], yB, rt[:, 2 * j + 1:2 * j + 2], ya, MUL, ADD)
                    nc.sync.dma_start(y_d[qt * 128:(qt + 1) * 128, :], yo[:])

            def drain_evac(c):
                """Emit the carried block's last o-chunk, then evacuate its
                oT rows + denominators. den2 rows 1:32 hold stale-but-finite
                data after the first two blocks, which the zero rows of
                `sel` ignore."""
                cA, cB, cpP, cp, cqc = (c["oA"], c["oB"], c["pP"], c["p"],
                                        c["qc"])
                _emit_o(nc, cA, cB, vaug_t, cpP, NPAIR - 1, cp,
                        start=False, stop=True)
                nc.vector.tensor_copy(oT[0:64, cp * SEQ + cqc * QCH:
                                         cp * SEQ + (cqc + 1) * QCH],
                                      cA[0:64, :])
                nc.vector.tensor_copy(oT[64:128, cp * SEQ + cqc * QCH:
                                         cp * SEQ + (cqc + 1) * QCH],
                                      cB[0:64, :])
                den2 = smallpool.tile([33, QCH], F32R, tag="den2")
                if cp == 0 and cqc < 2:
                    nc.vector.tensor_copy(den2[:], zden[:])
                nc.vector.tensor_copy(den2[0:1, :], cA[64:65, :])
                nc.vector.tensor_copy(den2[32:33, :], cB[64:65, :])
                return {"p": cp, "qc": cqc, "den2": den2}

            pending = None
            carry = None
            EXPF = mybir.ActivationFunctionType.Exp
            for p in range(2):
                Q = qkT_blk(2 * p)
                K = qkT_blk(2 * p + 1)
                for qc in range(NQC):
                    oA = oB = None   # allocated lazily at i==1 (after the
                    prev = None      # previous block's drain+evac released
                                     # the oA/oB banks)
                    for i in range(NPAIR):
                        # score quarters live in the persistent sps tile:
                        # [A kc | A kc+1 | B kc | B kc+1], one PSUM bank
                        # each. Head A's pair recycles banks 0:2 against
                        # exp0 only, head B's against exp1 (bank-granular
                        # WAR), which staggers PE refill against the ACT
                        # drain and keeps the exp stream gapless.
                        for hh in range(2):
                            for half in range(2):
                                kc = 2 * i + half
                                nc.tensor.matmul(
                                    sps[:, (2 * hh + half) * QCH:
                                        (2 * hh + half + 1) * QCH],
                                    _mm(K[64 * hh:64 * hh + 64,
                                          kc * 128:(kc + 1) * 128]),
                                    _mm(Q[64 * hh:64 * hh + 64,
                                          qc * QCH:(qc + 1) * QCH]),
                                    start=True, stop=True,
                                    skip_group_check=True,
                                )
                        # exps emitted directly after the score matmuls:
                        # the framework's engine-progress waits are coarse
                        # (count-at-emission), so anything emitted between
                        # the scores and the exps would falsely serialize
                        # the exp behind unrelated PE work
                        # bufs=4: with 3, exp(i)'s write-WAR lands on the
                        # o-matmuls of i-3 (still in flight one iteration
                        # ago); 4 pushes it back to the long-done i-4
                        pP = ptpool.tile([128, 4 * QCH], MMDT, tag="pP",
                                         bufs=4, name="pP")
                        nc.scalar.activation(pP[:, 0:2 * QCH],
                                             sps[:, 0:2 * QCH],
                                             EXPF, scale=SCALE, bias=embias[:])
                        nc.scalar.activation(pP[:, 2 * QCH:4 * QCH],
                                             sps[:, 2 * QCH:4 * QCH],
                                             EXPF, scale=SCALE, bias=embias[:])
                        if filler and p == 0:
                            filler.pop(0)()
                            if filler and qc == 0:
                                filler.pop(0)()
                        if i == 0 and carry is not None:
                            pending = drain_evac(carry)
                            carry = None
                        # previous block's out-projection, spread across
                        # this block's early pair-iterations
                        if pending is not None:
                            if i == 1:
                                emit_rt(pending)
                            elif 2 <= i < 2 + QCH // 128:
                                emit_yproj_j(pending, i - 2)
                                if i == 1 + QCH // 128:
                                    pending = None
                        if prev is not None:
                            if oA is None:
                                oA = ps.tile([65, QCH], F32, tag="oA",
                                             bufs=1, name="oA")
                                oB = ps.tile([65, QCH], F32, tag="oB",
                                             bufs=1, name="oB")
                            _emit_o(nc, oA, oB, vaug_t, prev, i - 1, p,
                                    start=(i == 1), stop=False)
                        prev = pP
                    # last o-chunk + evacuation drain into the next block's
                    # first iteration (keeps ACT fed across the boundary)
                    carry = {"oA": oA, "oB": oB, "pP": prev, "p": p,
                             "qc": qc}

            # tail: drain the last block + its out-projection
            pending = drain_evac(carry)
            emit_rt(pending)
            for j in range(QCH // 128):
                emit_yproj_j(pending, j, tail=True)

    nc.finalize()
    return nc


_NC_CACHE = {}


def get_nc():
    if "nc" not in _NC_CACHE:
        _NC_CACHE["nc"] = build_nc()
    return _NC_CACHE["nc"]


def make_core_inputs(x, w_qkv, w_out):
    """Per-core input dicts (host-side sharding)."""
    in_maps = []
    for c in range(NCORES):
        b, hg = c // 2, c % 2
        heads = [hg * HPC + i for i in range(HPC)]
        qcols = [w_qkv[:, h * HD:(h + 1) * HD] for h in heads]
        kcols = [w_qkv[:, DIM + h * HD:DIM + (h + 1) * HD] for h in heads]
        vcols = [w_qkv[:, 2 * DIM + h * HD:2 * DIM + (h + 1) * HD] for h in heads]
        wperm = np.concatenate(
            [qcols[0], qcols[1], kcols[0], kcols[1],
             qcols[2], qcols[3], kcols[2], kcols[3]], axis=1)
        wv = np.concatenate(vcols, axis=1)
        w2 = w_out[hg * HPC * HD:(hg + 1) * HPC * HD, :]
        import ml_dtypes
        mmnp = (ml_dtypes.bfloat16 if MMDT == mybir.dt.bfloat16
                else np.float16 if MMDT == mybir.dt.float16
                else np.float32)
        in_maps.append({
            "xt": np.ascontiguousarray(x[b].T).astype(mmnp),
            "wperm": np.ascontiguousarray(wperm).astype(mmnp),
            "wv": np.ascontiguousarray(wv).astype(mmnp),
            "w2": np.ascontiguousarray(w2).astype(mmnp),
        })
    return in_maps


def kernel(x, w_qkv, w_out, b_out):
    from concourse.bass_utils import run_bass_kernel_spmd

    x = np.asarray(x, dtype=np.float32)
    w_qkv = np.asarray(w_qkv, dtype=np.float32)
    w_out = np.asarray(w_out, dtype=np.float32)
    b_out = np.asarray(b_out, dtype=np.float32)

    nc = get_nc()
    in_maps = make_core_inputs(x, w_qkv, w_out)
    res = run_bass_kernel_spmd(nc, in_maps, list(range(NCORES))).results

    out = np.empty((B, SEQ, DIM), dtype=np.float32)
    for b in range(B):
        out[b] = (res[2 * b]["y"].astype(np.float32)
                  + res[2 * b + 1]["y"].astype(np.float32) + b_out)
    return out



# revision 16
# speedup vs baseline: 1.2268x; 1.2268x over previous
"""Multi-head self-attention forward on 8 Trainium2 NeuronCores.

Problem: x[4,2048,512] -> qkv proj (w_qkv [512,1536]) -> 8-head attention
(head_dim 64) -> out proj (w_out [512,512] + b_out) -> y[4,2048,512].

Sharding: 8 shards = (batch b in 0..3) x (head-group hg in 0..1, 4 heads each).
Core c handles b=c//2, hg=c%2. Each core computes, for its batch and its 4
heads: qkv projection (only its heads' columns), attention, and the partial
output projection restricted to its heads' rows of w_out. Host sums the two
half-projections per batch and adds the bias.

On-device layout (all "T" tensors keep the contraction dim on partitions):
  xT   [512, 2048]   x[b] transposed (host-side transpose)
  qkT  4 tiles [128, 2048]: Q01, K01, Q23, K23 (2 heads stacked per tile:
       head A on partitions 0:64, head B on 64:128)
  v_aug 16 seq-tiles [128, 4*65]: per head 64 v columns + a ones column
       (the ones column makes the oT matmul also produce the softmax
       denominator as row 64 of its output)
  sT   [k, q] scores transposed -> exp (no max subtraction: |s|~N(0,1), safe
       in fp32) -> pT
  oT   v_aug.T @ pT = [65, q]: rows 0:64 unnormalized head output (d on
       partitions), row 64 = softmax denominator
  yproj y[q,c] per head = oT_head.T @ w2_head, scaled per-partition (q) by
       1/denom via tensor_scalar, summed over the 4 heads on DVE.
"""

import numpy as np

import concourse.bass as bass
import concourse.mybir as mybir
import concourse.tile as tile
from concourse import bacc

DIM = 512
NHEADS = 8
HD = 64
B = 4
SEQ = 2048
SCALE = HD ** -0.5

NCORES = 8
HPC = 4          # heads per core
QCH = 512        # q chunk (moving free dim)
NQC = SEQ // QCH # 4 q-chunks
KCH = 128        # k chunk (psum partition dim)
NKC = SEQ // KCH # 16 k-chunks
CCH = 128        # contraction chunk for projections
NCC = DIM // CCH # 4

F32 = mybir.dt.float32
F32R = mybir.dt.float32r

BF16 = mybir.dt.bfloat16
F16 = mybir.dt.float16
# matmul input dtype. fp16: 1 cycle/row like bf16 (same FP22 PE path, FWL
# eligible), but 10 mantissa bits instead of 7 — buys precision headroom
# that the fp8 DoubleRow AV path below spends. All values on these paths
# (x, w, q, k ~ N(0,1); exp(s-2) <= 45; unnormalized o <= ~2500) sit well
# inside fp16 range.
MMDT = F16

# exp bias: all exps compute exp(s*SCALE - EXPB). Keeps the fp8 pP values
# of the DoubleRow iterations inside TRN-e4m3 range (max 240, inf at 256);
# the softmax ratio cancels the constant. Applied uniformly so the
# denominator stays consistent across chunks.
EXPB = -2.0

PREFILL = False


def _mm(ap):
    return ap


def _emit_o(nc, oA, oB, vaug_t, pP, i, p, start, stop):
    """Accumulate the two kc chunks of pair-iteration i into oA/oB.

    pP packs both heads' exp'd scores: [A(kc) A(kc+1) B(kc) B(kc+1)],
    each QCH wide."""
    QCH = 512
    for hh, odst in ((0, oA), (1, oB)):
        for half in range(2):
            kc = 2 * i + half
            c0 = (2 * hh + half) * QCH
            nc.tensor.matmul(
                odst[:],
                _mm(vaug_t(kc)[:, 2 * p + hh, :]),
                _mm(pP[:, c0:c0 + QCH]),
                start=(start and half == 0), stop=(stop and half == 1),
                skip_group_check=True,
            )


def build_nc():
    nc = bacc.Bacc()

    xT_d = nc.dram_tensor("xt", [DIM, SEQ], MMDT, kind="ExternalInput")
    wperm_d = nc.dram_tensor("wperm", [DIM, 4 * 128], MMDT, kind="ExternalInput")
    wv_d = nc.dram_tensor("wv", [DIM, HPC * HD], MMDT, kind="ExternalInput")
    w2_d = nc.dram_tensor("w2", [HPC * HD, DIM], MMDT, kind="ExternalInput")
    y_d = nc.dram_tensor("y", [SEQ, DIM], BF16, kind="ExternalOutput")

    with tile.TileContext(nc) as tc:
        with (
            tc.tile_pool(name="const", bufs=1) as cpool,
            tc.tile_pool(name="big", bufs=1) as bigpool,
            tc.tile_pool(name="pt", bufs=4) as ptpool,
            tc.tile_pool(name="yacc", bufs=1) as yaccpool,
            tc.tile_pool(name="tmp", bufs=3) as tmppool,
            tc.tile_pool(name="small", bufs=2) as smallpool,
            tc.tile_pool(name="ps", bufs=1, space="PSUM") as ps,
        ):
            # ---- constants / inputs to SBUF ----
            # DMA lands fp32 in a staging tile; a DVE copy rounds into the
            # f32r tile the matmuls read (BIR f32r-rounding requirement).
            xTs = [cpool.tile([128, SEQ], MMDT, tag=f"xT{c}", name=f"xT{c}")
                   for c in range(NCC)]
            wps = [cpool.tile([128, 512], MMDT, tag=f"wp{c}", name=f"wp{c}")
                   for c in range(NCC)]
            wvs = [cpool.tile([128, HPC * HD], MMDT, tag=f"wv{c}", name=f"wv{c}")
                   for c in range(NCC)]
            w2s = [cpool.tile([128, DIM], MMDT, tag=f"w2{p}", name=f"w2{p}")
                   for p in range(2)]
            wscr = cpool.tile([128, 512], MMDT, tag="wscr")
            nc.gpsimd.memset(wscr[:], 0.0)
            ones4 = cpool.tile([128, HPC], F32, tag="ones4")
            nc.gpsimd.memset(ones4[:], 1.0)
            ones1 = cpool.tile([1, 1], F32, tag="ones1")
            nc.gpsimd.memset(ones1[:], 1.0)
            # preload the exp ACT table set early so the first real exp in
            # the attention phase doesn't stall the pipeline ~2.7us (the
            # PE gap there is what re-throttles HAM to K=4/8)
            selst = cpool.tile([33, 2], F32, tag="selst")
            nc.gpsimd.memset(selst[:], 0.0)
            nc.gpsimd.memset(selst[0:1, 0:1], 1.0)
            nc.gpsimd.memset(selst[32:33, 1:2], 1.0)
            sel = cpool.tile([33, 2], F32R, tag="sel")
            nc.vector.tensor_copy(sel[:], selst[:])
            zden = cpool.tile([33, QCH], F32, tag="zden")
            nc.gpsimd.memset(zden[:], 0.0)
            embias = cpool.tile([128, 1], F32, tag="embias")
            nc.gpsimd.memset(embias[:], EXPB)
            dummy = cpool.tile([1, 1], F32, tag="dummy")
            nc.scalar.activation(dummy[:], ones1[:],
                                 mybir.ActivationFunctionType.Exp)
            # PE warmup during the input-DMA wait: junk matmuls keep the PE
            # busy so the HAM clock-gate reaches K=8/8 (2.4 GHz) before
            # phase 1, instead of gambling on the free-running HAM phase
            # (observed ~1-in-7 runs entering a cold-clock regime, +35us).
            # 20 junk matmuls (~8.5us at the cold 1.2GHz clock): enough to
            # keep the PE busy through the whole input-DMA window, so the
            # HAM activity monitor never sees an idle MID window between
            # warmup and phase 1 (which would re-throttle the clock and run
            # phase 1 at 1.2GHz).
            wups = ps.tile([4, 512], F32, tag="y", bufs=2, name="wups")
            for _ in range(20):
                nc.tensor.matmul(wups[:], wscr[:, 0:4], wscr[:],
                                 start=True, stop=True,
                                 skip_group_check=True)

            # Input DMAs ordered by first use: phase 1a accumulates over c,
            # so interleaving (wperm chunk c, xT chunk c) lets its first
            # matmul start after two transfers instead of eight; everything
            # else (wv for the v fillers, late wperm cols, w2) follows.
            # Input DMAs ordered by first use: phase 1a needs wperm cols
            # 0:256 + the leading xT seq halves; wv (v fillers), the late
            # xT/wperm cols and w2 follow. All on the hardware sync queue —
            # gpsimd's software DGE is too slow for anything load-bearing.
            for c in range(NCC):
                nc.sync.dma_start(wps[c][:, 0:256],
                                  wperm_d[c * 128:(c + 1) * 128, 0:256])
            # xT leading halves split into quarters: each DMA descriptor
            # runs ~40GB/s on one engine, so finer slices double the
            # engine-level parallelism on the phase-1a critical path (and
            # its first matmuls only need the first seq quarter).
            for c in range(NCC):
                nc.sync.dma_start(xTs[c][:, 0:512],
                                  xT_d[c * 128:(c + 1) * 128, 0:512])
            for c in range(NCC):
                nc.sync.dma_start(xTs[c][:, 512:1024],
                                  xT_d[c * 128:(c + 1) * 128, 512:1024])
            for c in range(NCC):
                nc.sync.dma_start(wvs[c][:], wv_d[c * 128:(c + 1) * 128, :])
            for c in range(NCC):
                nc.sync.dma_start(xTs[c][:, 1024:SEQ],
                                  xT_d[c * 128:(c + 1) * 128, 1024:SEQ])
            for c in range(NCC):
                nc.sync.dma_start(wps[c][:, 256:512],
                                  wperm_d[c * 128:(c + 1) * 128, 256:512])
            for p in range(2):
                nc.sync.dma_start(w2s[p][:], w2_d[p * 128:(p + 1) * 128, :])

            def xT_c(c):
                return xTs[c]

            # ---- persistent intermediates ----
            qkTs = [bigpool.tile([128, SEQ], MMDT, tag=f"qkT{m}",
                                 name=f"qkT{m}") for m in range(4)]
            vaugs = [bigpool.tile([128, HPC * 65], MMDT, tag=f"vaug{st}",
                                  name=f"vaug{st}") for st in range(NKC)]
            oT = bigpool.tile([128, 2 * SEQ], MMDT, tag="oT")    # pair-packed
            yacc = yaccpool.tile([128, SEQ // 128 * DIM], F32, tag="yacc")

            def qkT_blk(m):
                return qkTs[m]

            def vaug_t(kc):
                # [128, HPC, 65] view of seq-tile kc
                return vaugs[kc].rearrange("p (h e) -> p h e", e=65)

            # the ones columns of every vaug tile never change: write them
            # all once in the preamble (off the steady-state DVE path)
            for st in range(NKC):
                nc.vector.tensor_copy(
                    vaug_t(st)[:, :, 64:65],
                    ones4[:].rearrange("p (h o) -> p h o", o=1))

            def v_unit(st, tag, bufs):
                pv = ps.tile([128, HPC * HD], F32, tag=tag, bufs=bufs,
                             name="pv")
                for c in range(NCC):
                    nc.tensor.matmul(
                        pv[:],
                        _mm(xT_c(c)[:, st * 128:(st + 1) * 128]),
                        _mm(wvs[c][:]),
                        start=(c == 0), stop=(c == NCC - 1),
                        skip_group_check=True,
                    )
                vt = vaug_t(st)
                nc.vector.tensor_copy(
                    vt[:, :, 0:64], pv[:].rearrange("p (h d) -> p h d", d=HD)
                )

            def qk_unit(m, s2, tag, bufs):
                pp = ps.tile([128, 512], F32, tag=tag, bufs=bufs, name="pp")
                for c in range(NCC):
                    nc.tensor.matmul(
                        pp[:],
                        _mm(wps[c][:, m * 128:(m + 1) * 128]),
                        _mm(xT_c(c)[:, s2 * 512:(s2 + 1) * 512]),
                        start=(c == 0), stop=(c == NCC - 1),
                        skip_group_check=True,
                    )
                nc.vector.tensor_copy(qkTs[m][:, s2 * 512:(s2 + 1) * 512],
                                      pp[:])

            # ---- phase 1a (minimal serial prefix): Q01 for the first
            # q-block and K01 for the first half-seq; everything else
            # becomes attention filler so the exp stream starts as early as
            # possible ----
            # m=0 and m=1 use different PSUM tags so m1's matmuls don't
            # serialize behind m0's DVE evacuation copy (shared-bank WAR)
            for m, ptag, units in ((0, "sA", (0,)), (1, "sB", (0, 1))):
                pp = ps.tile([128, 1024], F32, tag=ptag, bufs=1, name="pp")
                for s2 in units:
                    for c in range(NCC):
                        nc.tensor.matmul(
                            pp[:, (s2 & 1) * 512:((s2 & 1) + 1) * 512],
                            _mm(wps[c][:, m * 128:(m + 1) * 128]),
                            _mm(xT_c(c)[:, s2 * 512:(s2 + 1) * 512]),
                            start=(c == 0),
                            stop=(c == NCC - 1),
                            skip_group_check=True,
                        )
                    nc.vector.tensor_copy(
                        qkT_blk(m)[:, s2 * 512:(s2 + 1) * 512],
                        pp[:, (s2 & 1) * 512:((s2 & 1) + 1) * 512])

            # ---- phase 1b: first two v seq-tiles; the rest are filler ----
            for st in range(2):
                v_unit(st, "y", 2)

            # Filler order is load-bearing: block (p0, qc0) pops two units
            # per iteration (before the carried o-drain at i==0), and the
            # schedule is checked against every consumer deadline:
            # scores(i) need K01 cols 256i:256i+256 (qk(1,s2) by i=2*s2-2)
            # and o(j), drained at i=j+1, needs vaug tiles 2j, 2j+1 by then.
            # qk(0,1) (Q01 cols 512:1024, first used by block qc=1) rides
            # mid-list in a slot with no earlier deadline pressure.
            filler = [lambda st=st: v_unit(st, "y", 2) for st in (2, 3, 4, 5)]
            filler += [lambda: qk_unit(1, 2, "y", 2)]
            filler += [lambda st=st: v_unit(st, "y", 2) for st in (6,)]
            filler += [lambda: qk_unit(0, 1, "y", 2)]
            filler += [lambda st=st: v_unit(st, "y", 2) for st in (7,)]
            filler += [lambda: qk_unit(1, 3, "y", 2)]
            filler += [lambda st=st: v_unit(st, "y", 2)
                       for st in range(8, NKC)]
            filler += [lambda s2=s2: qk_unit(0, s2, "y", 2) for s2 in (2, 3)]
            filler += [lambda m=m, s2=s2: qk_unit(m, s2, "y", 2)
                       for m in (2, 3) for s2 in range(4)]
            if PREFILL:
                while filler:
                    filler.pop(0)()

            # ---- phase 2: attention + out-proj ----
            # kc chunks processed in pairs: one s psum tile [128, 1024] holds
            # scores for kc and kc+1 side by side (the same 128 k-partitions
            # map to different k-blocks per column half; exp is elementwise)
            # halving ACT instruction count. Two levels of software
            # pipelining keep the PE stream dense: within a block, s(i+1) is
            # emitted before o(i) so the PE never head-of-line blocks on
            # exp(i); across blocks, the out-projection of block n is spread
            # into the first pair-iterations of block n+1.
            NPAIR = NKC // 2

            def emit_rt(pend):
                p, qc, den2 = pend["p"], pend["qc"], pend["den2"]
                rt_ps = ps.tile([128, 2 * (QCH // 128)], F32, tag="y",
                                bufs=2, name="rt_ps")
                for j in range(QCH // 128):
                    # [2,128] -> [128,2] transpose as a K=33 selector matmul
                    # (is_transpose matmuls crash the HW; K=1 f32r fails an
                    # ISA check; heads sit on partitions 0 and 32 of den2)
                    nc.tensor.matmul(
                        rt_ps[:, 2 * j:2 * j + 2],
                        den2[:, j * 128:(j + 1) * 128],
                        sel[:],
                        start=True, stop=True, skip_group_check=True,
                    )
                dt_sb = smallpool.tile([128, 2 * (QCH // 128)], F32,
                                       tag="dt_sb")
                nc.vector.tensor_copy(dt_sb[:], rt_ps[:])
                rt = smallpool.tile([128, 2 * (QCH // 128)], F32, tag="rt_sb")
                nc.vector.reciprocal(rt[:], dt_sb[:])
                pend["rt"] = rt

            def emit_yproj_j(pend, j, tail=False):
                # Fused multiply-accumulate evacuation: ya accumulates the 4
                # head-group contributions via scalar_tensor_tensor
                # (out = in0*rt + in1) entirely on DVE; the final op writes
                # the bf16 output tile that is DMA'd out.
                p, qc, rt = pend["p"], pend["qc"], pend["rt"]
                qt = qc * (QCH // 128) + j
                yA = ps.tile([128, DIM], F32, tag="y", bufs=2, name="yA")[:]
                yB = ps.tile([128, DIM], F32, tag="y", bufs=2, name="yB")[:]
                oTp = oT[:, p * SEQ:(p + 1) * SEQ]
                nc.tensor.matmul(
                    yA,
                    _mm(oTp[0:64, qt * 128:(qt + 1) * 128]),
                    _mm(w2s[p][0:64, :]),
                    start=True, stop=True, skip_group_check=True,
                )
                nc.tensor.matmul(
                    yB,
                    _mm(oTp[64:128, qt * 128:(qt + 1) * 128]),
                    _mm(w2s[p][64:128, :]),
                    start=True, stop=True, skip_group_check=True,
                )
                ya = yacc[:, qt * DIM:(qt + 1) * DIM]
                MUL = mybir.AluOpType.mult
                ADD = mybir.AluOpType.add
                if p == 0:
                    nc.vector.tensor_scalar_mul(
                        ya, yA, rt[:, 2 * j:2 * j + 1])
                    nc.vector.scalar_tensor_tensor(
                        ya, yB, rt[:, 2 * j + 1:2 * j + 2], ya, MUL, ADD)
                else:
                    nc.vector.scalar_tensor_tensor(
                        ya, yA, rt[:, 2 * j:2 * j + 1], ya, MUL, ADD)
                    yo = smallpool.tile([128, DIM], BF16, tag="yout")
                    nc.vector.scalar_tensor_tensor(
                        yo[:], yB, rt[:, 2 * j + 1:2 * j + 2], ya, MUL, ADD)
                    nc.sync.dma_start(y_d[qt * 128:(qt + 1) * 128, :], yo[:])

            def drain_evac(c):
                """Emit the carried block's last o-chunk, then evacuate its
                oT rows + denominators. den2 rows 1:32 hold stale-but-finite
                data after the first two blocks, which the zero rows of
                `sel` ignore."""
                cA, cB, cpP, cp, cqc = (c["oA"], c["oB"], c["pP"], c["p"],
                                        c["qc"])
                _emit_o(nc, cA, cB, vaug_t, cpP, NPAIR - 1, cp,
                        start=False, stop=True)
                nc.vector.tensor_copy(oT[0:64, cp * SEQ + cqc * QCH:
                                         cp * SEQ + (cqc + 1) * QCH],
                                      cA[0:64, :])
                nc.vector.tensor_copy(oT[64:128, cp * SEQ + cqc * QCH:
                                         cp * SEQ + (cqc + 1) * QCH],
                                      cB[0:64, :])
                den2 = smallpool.tile([33, QCH], F32R, tag="den2")
                if cp == 0 and cqc < 2:
                    nc.vector.tensor_copy(den2[:], zden[:])
                nc.vector.tensor_copy(den2[0:1, :], cA[64:65, :])
                nc.vector.tensor_copy(den2[32:33, :], cB[64:65, :])
                return {"p": cp, "qc": cqc, "den2": den2}

            pending = None
            carry = None
            EXPF = mybir.ActivationFunctionType.Exp
            for p in range(2):
                Q = qkT_blk(2 * p)
                K = qkT_blk(2 * p + 1)
                for qc in range(NQC):
                    oA = oB = None   # allocated lazily at i==1 (after the
                    prev = None      # previous block's drain+evac released
                                     # the oA/oB banks)
                    for i in range(NPAIR):
                        sA = ps.tile([128, 2 * QCH], F32, tag="sA", bufs=1,
                                     name="sA")
                        sB = ps.tile([128, 2 * QCH], F32, tag="sB", bufs=1,
                                     name="sB")
                        for hh, stile in ((0, sA), (1, sB)):
                            for half in range(2):
                                kc = 2 * i + half
                                nc.tensor.matmul(
                                    stile[:, half * QCH:(half + 1) * QCH],
                                    _mm(K[64 * hh:64 * hh + 64,
                                          kc * 128:(kc + 1) * 128]),
                                    _mm(Q[64 * hh:64 * hh + 64,
                                          qc * QCH:(qc + 1) * QCH]),
                                    start=True, stop=True,
                                    skip_group_check=True,
                                )
                        # exps emitted directly after the score matmuls:
                        # the framework's engine-progress waits are coarse
                        # (count-at-emission), so anything emitted between
                        # the scores and the exps would falsely serialize
                        # the exp behind unrelated PE work
                        # bufs=4: with 3, exp(i)'s write-WAR lands on the
                        # o-matmuls of i-3 (still in flight one iteration
                        # ago); 4 pushes it back to the long-done i-4
                        pP = ptpool.tile([128, 4 * QCH], MMDT, tag="pP",
                                         bufs=4, name="pP")
                        nc.scalar.activation(pP[:, 0:2 * QCH], sA[:],
                                             EXPF, scale=SCALE,
                                             bias=embias[:])
                        nc.scalar.activation(pP[:, 2 * QCH:4 * QCH],
                                             sB[:],
                                             EXPF, scale=SCALE,
                                             bias=embias[:])
                        if filler and p == 0:
                            filler.pop(0)()
                            if filler and qc == 0:
                                filler.pop(0)()
                        if i == 0 and carry is not None:
                            pending = drain_evac(carry)
                            carry = None
                        # previous block's out-projection, spread across
                        # this block's early pair-iterations
                        if pending is not None:
                            if i == 1:
                                emit_rt(pending)
                            elif 2 <= i < 2 + QCH // 128:
                                emit_yproj_j(pending, i - 2)
                                if i == 1 + QCH // 128:
                                    pending = None
                        if prev is not None:
                            if oA is None:
                                oA = ps.tile([65, QCH], F32, tag="oA",
                                             bufs=1, name="oA")
                                oB = ps.tile([65, QCH], F32, tag="oB",
                                             bufs=1, name="oB")
                            _emit_o(nc, oA, oB, vaug_t, prev, i - 1, p,
                                    start=(i == 1), stop=False)
                        prev = pP
                    # last o-chunk + evacuation drain into the next block's
                    # first iteration (keeps ACT fed across the boundary)
                    carry = {"oA": oA, "oB": oB, "pP": prev, "p": p,
                             "qc": qc}

            # tail: drain the last block + its out-projection
            pending = drain_evac(carry)
            emit_rt(pending)
            for j in range(QCH // 128):
                emit_yproj_j(pending, j, tail=True)

    nc.finalize()
    return nc


_NC_CACHE = {}


def get_nc():
    if "nc" not in _NC_CACHE:
        _NC_CACHE["nc"] = build_nc()
    return _NC_CACHE["nc"]


def make_core_inputs(x, w_qkv, w_out):
    """Per-core input dicts (host-side sharding)."""
    in_maps = []
    for c in range(NCORES):
        b, hg = c // 2, c % 2
        heads = [hg * HPC + i for i in range(HPC)]
        qcols = [w_qkv[:, h * HD:(h + 1) * HD] for h in heads]
        kcols = [w_qkv[:, DIM + h * HD:DIM + (h + 1) * HD] for h in heads]
        vcols = [w_qkv[:, 2 * DIM + h * HD:2 * DIM + (h + 1) * HD] for h in heads]
        wperm = np.concatenate(
            [qcols[0], qcols[1], kcols[0], kcols[1],
             qcols[2], qcols[3], kcols[2], kcols[3]], axis=1)
        wv = np.concatenate(vcols, axis=1)
        w2 = w_out[hg * HPC * HD:(hg + 1) * HPC * HD, :]
        import ml_dtypes
        mmnp = (ml_dtypes.bfloat16 if MMDT == mybir.dt.bfloat16
                else np.float16 if MMDT == mybir.dt.float16
                else np.float32)
        in_maps.append({
            "xt": np.ascontiguousarray(x[b].T).astype(mmnp),
            "wperm": np.ascontiguousarray(wperm).astype(mmnp),
            "wv": np.ascontiguousarray(wv).astype(mmnp),
            "w2": np.ascontiguousarray(w2).astype(mmnp),
        })
    return in_maps


def kernel(x, w_qkv, w_out, b_out):
    from concourse.bass_utils import run_bass_kernel_spmd

    x = np.asarray(x, dtype=np.float32)
    w_qkv = np.asarray(w_qkv, dtype=np.float32)
    w_out = np.asarray(w_out, dtype=np.float32)
    b_out = np.asarray(b_out, dtype=np.float32)

    nc = get_nc()
    in_maps = make_core_inputs(x, w_qkv, w_out)
    res = run_bass_kernel_spmd(nc, in_maps, list(range(NCORES))).results

    out = np.empty((B, SEQ, DIM), dtype=np.float32)
    for b in range(B):
        out[b] = (res[2 * b]["y"].astype(np.float32)
                  + res[2 * b + 1]["y"].astype(np.float32) + b_out)
    return out

